# revision 20
# baseline (speedup 1.0000x reference)
# Trainium2 Bass kernel for nn_ContinuousHopfieldNet_70652212019686.
#
# Math (verified numerically against the jax reference):
#   B[i,:] = (k[4i] + k[4i+1] + k[4i+2] + k[4i+3]) / 4.5        (nb x d)
#   per retrieval iteration:
#     S = q @ B.T ; m = max(rowmax(S), 0) ; E = exp(S - m)
#     Z = E @ wbin + w_none * exp(-m) ; q' = (E @ (wbin*B)) / Z
#
# v3 sharding: the harness exec window is dominated by INPUT UPLOAD (the v1
# baseline replicated the 16MB k to all 8 cores: 132.6MB ~ 3.1ms at PCIe
# ~43GB/s, while the device body is only ~110us).  So k is sharded: core c
# uploads rows [512c, 512c+512) (2MB), computes its 128-bin binsum slice
# (512KB f32), and one DRAM AllGather (4MB out, ~0.3ms in this stack: ~185us
# fixed + ~27us/MB) replicates the full raw binsum.  Each core then builds
# the bf16 basis (Bw scaled, BT transposed split hi/lo) fused chunk-by-chunk
# into retrieval iteration 1, exactly like v1.  Upload total: ~20.2MB -- the
# floor without moving input-dependent math to the host.
#
# Precision plan (inherited from v1, validated):
#   - iter-1 S: 3-term split-bf16 (Qh@BTh + Ql@BTh + Qh@BTl).
#   - iter-2/3 S: plain bf16; U = E@Bw plain bf16; E plain bf16; Z uses the
#     same truncated E so the leading-order E error cancels in U/Z.
#   - iter-1 needs NO max subtraction (raw scores <= ~324, exp(s/4.5) fits).
#   - basis matmuls use RAW binsums; the 1/4.5 rides the exp's scale.
#   - iters hand over U TRANSPOSED (= next S's lhsT layout) and UNNORMALIZED
#     (1/Z rides the next exp's per-partition scale).
import numpy as np

NB = 1024
D = 1024
KLEN = 4096
NQ = 1024
NPTS = 2048
NCORES = 8
QS = NQ // NCORES
KS = KLEN // NCORES
NITER = 3

MM_DTYPE = "bf16-plan-v3-allgather4mb"  # informational


def _host_constants():
    """Input-independent basis constants, replicating reference fp32 math.

    Verified bit-identical to the jax reference in test.py."""
    t = np.linspace(0.0, 1.0, NPTS).astype(np.float32)
    dt = np.diff(t)
    w = np.concatenate([dt[:1] / 2, (dt[:-1] + dt[1:]) / 2, dt[-1:] / 2]).astype(
        np.float32
    )
    edges = (np.arange(NB + 1, dtype=np.float64) / NB).astype(np.float32)
    lb, ub = edges[:-1], edges[1:]
    cand = np.clip(np.searchsorted(ub, t, side="right"), 0, NB - 1)
    ok = (t >= lb[cand]) & (t < ub[cand])
    wbin64 = np.zeros(NB)
    np.add.at(wbin64, cand[ok], w[ok].astype(np.float64))
    wbin = wbin64.astype(np.float32)
    w_none = float(w[~ok].astype(np.float64).sum())
    # [128, 8] per-(partition, bin-chunk) layouts: wzc[p, c] = wbin[128c + p]
    wzc = wbin.reshape(8, 128).T.copy()
    wdiv = (wzc * np.float32(1.0 / 4.5)).astype(np.float32)
    wz = np.zeros((128, 8, 2), np.float32)  # N=2 pad for the Z matmul
    wz[:, :, 0] = wzc
    return wz, wdiv, w_none


def _build_program(bench_trips=0, bench_scope="full", ks_internal=False, bench_no_cc=False):
    import concourse.bacc as bacc
    import concourse.tile as tile
    from concourse import mybir
    from concourse.masks import make_identity

    F32 = mybir.dt.float32
    BF16 = mybir.dt.bfloat16
    SC = float(1.0 / 4.5)

    _, _, w_none = _host_constants()
    ln_wnone = float(np.log(np.float64(w_none)))

    nc = bacc.Bacc(
        "TRN2",
        target_bir_lowering=False,
        debug=False,
        enable_asserts=True,
        num_devices=NCORES,
    )
    ks_kind = "Internal" if ks_internal else "ExternalInput"
    ks = nc.dram_tensor("ks", [KS, D], F32, kind=ks_kind).ap()
    qs = nc.dram_tensor("qs", [QS, D], F32, kind="ExternalInput").ap()
    wz_d = nc.dram_tensor("wz", [128, 8, 2], F32, kind="ExternalInput").ap()
    wdiv_d = nc.dram_tensor("wdiv", [128, 8], F32, kind="ExternalInput").ap()
    out_d = nc.dram_tensor("out", [QS, D], F32, kind="ExternalOutput").ap()
    # collective: raw per-core binsum slice (f32) -> full 1024-bin binsum
    payload = nc.dram_tensor("payload", [128, D], F32, kind="Internal").ap()
    gathered = nc.dram_tensor(
        "gathered", [NCORES, 128, D], F32, kind="Internal", addr_space="Shared"
    ).ap()

    with tile.TileContext(nc) as tc:
        with (
            tc.tile_pool(name="const", bufs=1) as constp,
            tc.tile_pool(name="ksrc", bufs=1) as kpool,
            tc.tile_pool(name="bsrc", bufs=3) as bpool,
            tc.tile_pool(name="work", bufs=2) as work,
            tc.tile_pool(name="iterp", bufs=2) as iterp,
            tc.tile_pool(name="stats", bufs=4) as stats,
            tc.tile_pool(name="psA", bufs=1, space="PSUM") as psA,  # U/UT accum
            tc.tile_pool(name="psB", bufs=1, space="PSUM") as psB,  # S
            tc.tile_pool(name="psT", bufs=1, space="PSUM") as psT,  # f32 transposes
            tc.tile_pool(name="psTb", bufs=2, space="PSUM") as psTb,  # bf16 transposes
            tc.tile_pool(name="psZ", bufs=1, space="PSUM") as psZ,  # Z accum
        ):
            ident = constp.tile([128, 128], F32)
            make_identity(nc, ident)
            ident_bf = constp.tile([128, 128], BF16)
            nc.vector.tensor_copy(ident_bf, ident)
            wn_sb = constp.tile([128, 1], F32)
            nc.vector.memset(wn_sb, w_none)
            lnw_sb = constp.tile([128, 1], F32)
            nc.vector.memset(lnw_sb, ln_wnone)
            ones_bf = constp.tile([1, 128], BF16)
            nc.vector.memset(ones_bf, 1.0)
            wz_sb = constp.tile([128, 8, 2], F32)
            nc.sync.dma_start(wz_sb, wz_d)
            wdiv_sb = constp.tile([128, 8], F32)
            nc.sync.dma_start(wdiv_sb, wdiv_d)
            wz_hi = constp.tile([128, 8, 2], BF16)
            nc.vector.tensor_copy(wz_hi, wz_sb)
            wz_lo = constp.tile([128, 8, 2], BF16)
            nc.vector.tensor_tensor(wz_lo, wz_sb, wz_hi, mybir.AluOpType.subtract)

            # full-basis weights, persistent across iterations
            Bw_hi = constp.tile([128, 8, D], BF16, tag="Bw_hi")
            BT_hi = constp.tile([128, 8, NB], BF16, tag="BT_hi")
            BT_lo = constp.tile([128, 8, NB], BF16, tag="BT_lo")

            def binsum_and_gather():
                """This core's 128-bin binsum slice -> AllGather full binsum."""
                kt = kpool.tile([128, 4, D], F32, tag="kt")
                nc.sync.dma_start(kt, ks.rearrange("(p r) d -> p r d", r=4))
                a1 = work.tile([128, D], F32, tag="a1")
                nc.vector.tensor_add(a1, kt[:, 0], kt[:, 1])
                a2 = work.tile([128, D], F32, tag="a2")
                nc.gpsimd.tensor_add(a2, kt[:, 2], kt[:, 3])
                bsum = work.tile([128, D], F32, tag="bsum")
                nc.vector.tensor_add(bsum, a1, a2)
                nc.sync.dma_start(payload, bsum)
                if not bench_no_cc:
                    nc.gpsimd.collective_compute(
                        "AllGather",
                        mybir.AluOpType.bypass,
                        replica_groups=[list(range(NCORES))],
                        ins=[payload],
                        outs=[gathered],
                    )

            def build_q0():
                """Qt1 hi/lo: Qt[p, kd, j] = q[j, 128 kd + p], split bf16."""
                qn = work.tile([128, D], F32, tag="qn")
                nc.sync.dma_start(qn, qs)
                Qt_hi = iterp.tile([128, 8, QS], BF16, tag="qt_hi")
                Qt_lo = iterp.tile([128, 8, QS], BF16, tag="qt_lo", name="qt_lo")
                for h in range(2):
                    pt4 = psT.tile([128, 512], F32, tag="pt4")
                    for j in range(4):
                        kd = 4 * h + j
                        nc.tensor.transpose(
                            pt4[:, 128 * j : 128 * (j + 1)],
                            qn[:, 128 * kd : 128 * (kd + 1)],
                            ident,
                        )
                    pv = pt4.rearrange("p (a b) -> p a b", a=4)
                    nc.scalar.copy(Qt_hi[:, 4 * h : 4 * h + 4, :], pv)
                    nc.vector.tensor_tensor(
                        Qt_lo[:, 4 * h : 4 * h + 4, :],
                        pv,
                        Qt_hi[:, 4 * h : 4 * h + 4, :],
                        mybir.AluOpType.subtract,
                    )
                return Qt_hi, Qt_lo

            def transpose_E(E, ET, blocks):
                """ET[:, c] = E[:, 128c:128(c+1)].T for c in blocks (bf16).
                blocks must be contiguous runs aligned to the ET layout."""
                for h in range(0, len(blocks), 4):
                    grp = blocks[h : h + 4]
                    ptb = psTb.tile([128, 512], BF16, tag="ptb")
                    for j, c in enumerate(grp):
                        nc.tensor.transpose(
                            ptb[:, 128 * j : 128 * (j + 1)],
                            E[:, 128 * c : 128 * (c + 1)],
                            ident_bf,
                        )
                    pv = ptb[:, : 128 * len(grp)].rearrange(
                        "p (a b) -> p a b", a=len(grp)
                    )
                    nc.vector.tensor_copy(ET[:, grp[0] : grp[0] + len(grp), :], pv)

            def st_chunk(S, c, lhs_terms, sqT=None):
                """S[:, c] (transposed scores [bin, q] for bin block c) =
                sum over (BT term, Qt term) pairs and kd of
                BTx[:, kd, cs].T @ Qtx[:, kd].  Swapping lhs/rhs relative to
                the plain S = q @ B.T gives scores TRANSPOSED, so E lands
                directly in the [bin, q] layout Z/U need -- no E transposes.

                sqT ([1, 128] bf16, -4.5*||q'||^2 per query) adds a rank-1
                per-query shift via one k=1 matmul: a softmax max-surrogate
                that works in the transposed layout.  Provably
                0 <= max_j s_j - 4.5||q'||^2 <= ~max||binsum||^2/18 (~60/4.5
                exp-units), since q' is a sub-convex combination of
                binsum/4.5 rows, so exp never overflows and the leading term
                never underflows.  (Iter-1's raw q is bounded instead:
                scores <= ~324, exp(s/4.5) <= e^72 fits f32/bf16.)"""
                cs = slice(128 * c, 128 * (c + 1))
                n_mm = len(lhs_terms) * 8 + (1 if sqT is not None else 0)
                i_mm = 0
                for bt, qt in lhs_terms:
                    for kd in range(8):
                        nc.tensor.matmul(
                            S[:, c],
                            bt[:, kd, cs],
                            qt[:, kd],
                            start=(i_mm == 0),
                            stop=(i_mm == n_mm - 1),
                        )
                        i_mm += 1
                if sqT is not None:
                    nc.tensor.matmul(
                        S[:, c],
                        ones_bf,
                        sqT,
                        start=False,
                        stop=True,
                    )

            def accum_Z(Z, ET, c, first, last):
                nc.tensor.matmul(Z, ET[:, c], wz_hi[:, c], start=first, stop=False)
                nc.tensor.matmul(Z, ET[:, c], wz_lo[:, c], start=False, stop=last)

            def accum_U(U, ET, c, first, last):
                """U[q-part, d] += ET[:, c].T @ Bw[:, c] (one accumulation
                group per 512-wide PSUM bank region)."""
                for n in range(2):
                    ns = slice(512 * n, 512 * (n + 1))
                    nc.tensor.matmul(
                        U[:, ns], ET[:, c], Bw_hi[:, c, ns], start=first, stop=last
                    )

            def handover(U, rc):
                """q' = U/Z in bf16 (the per-query 1/Z is a per-partition
                scale HERE, pre-transpose), transposed into the next
                iteration's rhs layout [d, q].  Also emits the per-query
                shift row sqT[0, q] = bf16(-4.5*||q'_q||^2) and the matching
                none-bin normalizer zc_q = w_none * exp(sqT_q / 4.5), both
                derived from the SAME bf16 value so the shift cancels
                exactly."""
                qb = iterp.tile([128, D], BF16, tag="qb")
                nc.scalar.mul(qb, U, rc)
                QtU = iterp.tile([128, 8, QS], BF16, tag="qt_hi")
                transpose_E(qb, QtU, list(range(8)))
                q2 = work.tile([128, D], F32, tag="q2")
                nc.vector.tensor_tensor(q2, qb, qb, mybir.AluOpType.mult)
                sq = stats.tile([128, 1], F32, tag="sq")
                nc.vector.reduce_sum(sq, q2, axis=mybir.AxisListType.X)
                sqm = stats.tile([128, 1], BF16, tag="sqm")
                nc.vector.tensor_scalar_mul(sqm, sq, -4.5)
                ptb = psTb.tile([128, 512], BF16, tag="ptb")
                nc.tensor.transpose(ptb[:1, :128], sqm, ident_bf)
                sqT = stats.tile([1, 128], BF16, tag="sqT")
                nc.scalar.copy(sqT, ptb[:1, :128])
                zc = stats.tile([128, 1], F32, tag="zc")
                nc.scalar.activation(
                    zc,
                    sqm,
                    mybir.ActivationFunctionType.Exp,
                    scale=SC,
                    bias=lnw_sb[:, :1],
                )
                return QtU, sqT, zc

            def build_and_iter1(Qt_hi, Qt_lo):
                """Basis build from the gathered binsum fused with iter-1
                (no-max softmax, transposed scores), chunk by chunk."""
                ET1 = iterp.tile([128, 8, QS], BF16, tag="ET")
                U1 = psA.tile([128, D], F32, tag="U")
                S1 = psB.tile([128, 8, QS], F32, tag="S")
                Z1 = psZ.tile([128, 2], F32, tag="Z")
                for c in range(8):
                    bs = bpool.tile([128, D], F32, tag="bs")
                    nc.sync.dma_start(bs, gathered[c])
                    # Bw chunk: wbin/4.5-scaled binsum (bf16)
                    nc.scalar.mul(Bw_hi[:, c], bs, wdiv_sb[:, c : c + 1])
                    # BT chunk: transpose + split hi/lo
                    for h in range(2):
                        pt4 = psT.tile([128, 512], F32, tag="pt4")
                        for j in range(4):
                            kd = 4 * h + j
                            nc.tensor.transpose(
                                pt4[:, 128 * j : 128 * (j + 1)],
                                bs[:, 128 * kd : 128 * (kd + 1)],
                                ident,
                            )
                        pv = pt4.rearrange("p (a b) -> p a b", a=4)
                        cs = slice(128 * c, 128 * (c + 1))
                        nc.scalar.copy(BT_hi[:, 4 * h : 4 * h + 4, cs], pv)
                        nc.vector.tensor_tensor(
                            BT_lo[:, 4 * h : 4 * h + 4, cs],
                            pv,
                            BT_hi[:, 4 * h : 4 * h + 4, cs],
                            mybir.AluOpType.subtract,
                        )
                    # iter-1 transposed S for this bin block: 3-term split.
                    # The two BT_hi terms are issued first so they can start
                    # before the BT_lo subtract lands.
                    st_chunk(
                        S1,
                        c,
                        [(BT_hi, Qt_hi), (BT_hi, Qt_lo), (BT_lo, Qt_hi)],
                    )
                    nc.scalar.activation(
                        ET1[:, c],
                        S1[:, c],
                        mybir.ActivationFunctionType.Exp,
                        scale=SC,
                    )
                    accum_Z(Z1, ET1, c, first=(c == 0), last=(c == 7))
                    accum_U(U1, ET1, c, first=(c == 0), last=(c == 7))
                # rc1 = 1 / (Z1 + w_none); folded into the handover q'=U/Z
                zf = stats.tile([128, 1], F32, tag="zf")
                nc.vector.tensor_add(zf, Z1[:, 0:1], wn_sb)
                rc = stats.tile([128, 1], F32, tag="rc")
                nc.vector.reciprocal(rc, zf)
                return handover(U1, rc)

            def iter23(QtU, sqT, zc, last):
                """S (transposed) from the normalized q' rhs, with the
                rank-1 ||q'||^2 shift standing in for the row-max.
                Chunk-pipelined: each bin block's E/Z/U starts as soon as
                its 9 S matmuls finish.  Returns (QtU', sqT', zc') or
                writes the output."""
                S = psB.tile([128, 8, QS], F32, tag="S")
                ET = iterp.tile([128, 8, QS], BF16, tag="ET")
                Z = psZ.tile([128, 2], F32, tag="Z")
                U = psA.tile([128, D], F32, tag="U")
                for c in range(8):
                    st_chunk(S, c, [(BT_hi, QtU)], sqT=sqT)
                    nc.scalar.activation(
                        ET[:, c],
                        S[:, c],
                        mybir.ActivationFunctionType.Exp,
                        scale=SC,
                    )
                    accum_Z(Z, ET, c, first=(c == 0), last=(c == 7))
                    accum_U(U, ET, c, first=(c == 0), last=(c == 7))
                zf = stats.tile([128, 1], F32, tag="zf")
                nc.vector.tensor_add(zf, Z[:, 0:1], zc)
                rc = stats.tile([128, 1], F32, tag="rc")
                nc.vector.reciprocal(rc, zf)
                if last:
                    un = iterp.tile([128, D], F32, tag="un")
                    nc.scalar.mul(un, U, rc)
                    nc.sync.dma_start(out_d, un)
                    return None
                return handover(U, rc)

            def body():
                binsum_and_gather()
                Qt_hi, Qt_lo = build_q0()
                QtU, sqT, zc = build_and_iter1(Qt_hi, Qt_lo)
                QtU, sqT, zc = iter23(QtU, sqT, zc, last=False)
                iter23(QtU, sqT, zc, last=True)

            if bench_trips and bench_scope == "build":
                with tc.For_i(0, bench_trips, 1):
                    binsum_and_gather()
                    build_q0()
            elif bench_trips and bench_scope == "iters":
                binsum_and_gather()
                Qt_hi, Qt_lo = build_q0()
                with tc.For_i(0, bench_trips, 1):
                    QtU, sqT, zc = build_and_iter1(Qt_hi, Qt_lo)
                    r2 = iter23(QtU, sqT, zc, last=False)
                    iter23(*r2, last=True)
            elif bench_trips:
                # "full" / "full_nocc" (collective skipped via bench_no_cc)
                with tc.For_i(0, bench_trips, 1):
                    body()
            else:
                body()

    nc.compile()
    return nc


_CACHE = {}
LAST_RESULTS = None


def kernel(**inputs):
    global LAST_RESULTS
    k = np.ascontiguousarray(np.asarray(inputs["k"], dtype=np.float32))
    q = np.ascontiguousarray(np.asarray(inputs["q"], dtype=np.float32))
    assert k.shape == (KLEN, D) and q.shape == (NQ, D)

    if "nc" not in _CACHE:
        _CACHE["nc"] = _build_program()
        _CACHE["consts"] = _host_constants()
    nc = _CACHE["nc"]
    wz, wdiv, _ = _CACHE["consts"]

    in_maps = []
    for c in range(NCORES):
        in_maps.append(
            {
                "ks": np.ascontiguousarray(k[KS * c : KS * (c + 1)]),
                "qs": np.ascontiguousarray(q[QS * c : QS * (c + 1)]),
                "wz": wz,
                "wdiv": wdiv,
            }
        )

    import concourse.bass_utils as bass_utils

    res = bass_utils.run_bass_kernel_spmd(nc, in_maps, core_ids=list(range(NCORES)))
    LAST_RESULTS = res
    out = np.concatenate([res.results[c]["out"] for c in range(NCORES)], axis=0)
    return np.ascontiguousarray(out, dtype=np.float32)


if __name__ == "__main__":
    rng = np.random.default_rng(0)
    k = rng.standard_normal((KLEN, D), dtype=np.float32)
    q = rng.standard_normal((NQ, D), dtype=np.float32)
    o = kernel(k=k, q=q)
    print("kernel ran, out shape", o.shape, "finite:", np.isfinite(o).all())
